# revision 35
# baseline (speedup 1.0000x reference)
"""Bass/Trainium2 SPMD kernel for nn_BlockCausalTransformer_884763263652.

Sharding over 8 NeuronCores (one chip):
  - Residual x sequence-sharded: core c owns rows [c*256, (c+1)*256).
  - Attention head-sharded: core c owns heads (2c, 2c+1); q/k/v/mix built
    from a column shard of w_qkv (+ w_mix), attention runs fully local.
  - FFN hidden-sharded: hidden padded 2730 -> 2816 = 8*352; core c owns
    slice [c*352,(c+1)*352) of both the a and g halves.
  - Communication per layer: AllGather of h^T (rmsnorm output, bf16) for
    the qkv / ff_in matmuls, and AllToAll of o^T and act^T, which hands
    every core exactly the [contraction, its-rows] slice it needs as the
    stationary operand of the output projections - no reduce collectives
    and no partial sums ever cross cores.
  - Attention is computed in the sim^T = k^T.T @ q^T orientation with an
    extra ones-column appended to v, so softmax needs no transposes and
    the denominator falls out of the same matmul chain.
RMSNorm weights are folded host-side into the following matmul weights;
final_norm_w is applied host-side to the gathered output.
"""

import sys

sys.path.insert(0, "/opt/trn_rl_repo")

import numpy as np
import ml_dtypes

import concourse.bacc as bacc
import concourse.tile as tile
import concourse.mybir as mybir
from concourse.bass_utils import run_bass_kernel_spmd

F32 = mybir.dt.float32
BF16 = mybir.dt.bfloat16
AF = mybir.ActivationFunctionType

DEPTH = 4
DIM = 1024
HEADS = 16
DIM_HEAD = 64
FF_HID = 2730
N = 2048
EPS = 1.1920929e-07
SCALE = DIM_HEAD ** -0.5
N_CORES = 8

ROWS = N // N_CORES            # 256 sequence rows per core
H_PER = HEADS // N_CORES       # 2 heads per core
KD = DIM // 128                # 8 feature chunks
KF = (FF_HID + 127) // 128     # 22 hidden chunks (last ragged: 42 rows)
MI_IN = 2 * KF                 # 44 ff_in M-chunks: a chunks then g chunks
NJ = N // 512                  # 4 sequence 512-chunks
QC = N // 128                  # 16 kpos/qpos 128-chunks
W_QKVM = 3 * H_PER * DIM_HEAD + H_PER  # 386 cols: q|k|v (128 each) + mix (2)
FF_ROWS = [min(128, FF_HID - k * 128) for k in range(KF)]
WI_C = 2                       # ff_in weight stream chunk (column-blocks)
WF_C = 6                       # ff_out weight stream chunk (column-blocks)

_BF = ml_dtypes.bfloat16
DEBUG_TAPS = False


def _bf16(a):
    return np.ascontiguousarray(a.astype(_BF))


def _local_ag(nc, src, dst, n):
    for r in range(n):
        nc.sync.dma_start(dst[r * src.shape[0]:(r + 1) * src.shape[0], :],
                          src[:])


def _build_bass(local_comm=False, n_devices=N_CORES):
    nc = bacc.Bacc("TRN2", target_bir_lowering=False, debug=False,
                   num_devices=n_devices)

    # ---- I/O ----
    x_in = nc.dram_tensor("x", [ROWS, DIM], F32, kind="ExternalInput").ap()
    out_ext = nc.dram_tensor("out", [ROWS, DIM], F32, kind="ExternalOutput").ap()
    wqkvm = nc.dram_tensor("wqkvm", [DEPTH, 128, KD, W_QKVM], BF16,
                           kind="ExternalInput").ap()
    bmix = nc.dram_tensor("bmix", [DEPTH, H_PER, 1], F32,
                          kind="ExternalInput").ap()
    w_in = nc.dram_tensor("w_in", [DEPTH, MI_IN, 128, KD, 128], BF16,
                          kind="ExternalInput").ap()
    b_in = nc.dram_tensor("b_in", [DEPTH, 128, MI_IN], F32,
                          kind="ExternalInput").ap()
    w_o = nc.dram_tensor("w_o", [DEPTH, 128, KD, DIM], BF16,
                         kind="ExternalInput").ap()
    w_ffo = nc.dram_tensor("w_ffo", [DEPTH, 128, KF, DIM], BF16,
                           kind="ExternalInput").ap()
    b_ffo = nc.dram_tensor("b_ffo", [DEPTH, 1, DIM], BF16,
                           kind="ExternalInput").ap()
    ropes = nc.dram_tensor("ropes", [2, 128, N], BF16,
                           kind="ExternalInput").ap()
    mt_in = nc.dram_tensor("mt", [128, 128], BF16, kind="ExternalInput").ap()
    id_in = nc.dram_tensor("ident", [128, 128], BF16, kind="ExternalInput").ap()
    if DEBUG_TAPS:
        dbg_qr = nc.dram_tensor("dbg_qr", [128, N], BF16,
                                kind="ExternalOutput").ap()
        dbg_kr = nc.dram_tensor("dbg_kr", [128, N], BF16,
                                kind="ExternalOutput").ap()
        dbg_oT = nc.dram_tensor("dbg_oT", [128, N], BF16,
                                kind="ExternalOutput").ap()
        dbg_x1 = nc.dram_tensor("dbg_x1", [128, 2 * DIM], F32,
                                kind="ExternalOutput").ap()

    # ---- per-layer DRAM bounce buffers for collectives ----
    rg = [list(range(N_CORES))]
    bh_in = [nc.dram_tensor(f"hin{j}", [DIM, ROWS], BF16).ap()
             for j in range(DEPTH)]
    bh_out = [nc.dram_tensor(f"hout{j}", [N_CORES * DIM, ROWS], BF16,
                             addr_space="Shared").ap()
              for j in range(DEPTH)]
    bo_in = [nc.dram_tensor(f"oin{i}", [N_CORES * 128, ROWS], BF16).ap()
             for i in range(DEPTH)]
    bo_out = [nc.dram_tensor(f"oout{i}", [N_CORES * 128, ROWS], BF16).ap()
              for i in range(DEPTH)]

    with tile.TileContext(nc) as tc:
        with (
            tc.tile_pool(name="persist", bufs=1) as pp,
            tc.tile_pool(name="wq", bufs=2) as wqp,
            tc.tile_pool(name="wi", bufs=6) as wip,
            tc.tile_pool(name="wo", bufs=1) as wop,
            tc.tile_pool(name="wf", bufs=6) as wfp,
            tc.tile_pool(name="hT", bufs=1) as hp,
            tc.tile_pool(name="qk", bufs=1) as qkp,
            tc.tile_pool(name="attn", bufs=6) as ap_,
            tc.tile_pool(name="scratch", bufs=2) as sp,
            tc.tile_pool(name="stage", bufs=1) as stp,
            tc.tile_pool(name="slice", bufs=1) as slp,
            tc.tile_pool(name="mm", bufs=2, space="PSUM") as mmp,
            tc.tile_pool(name="ps2", bufs=3, space="PSUM") as psp,
        ):
            # persistent tiles
            x_sb = pp.tile([128, 2, DIM], F32, name="x_sb")
            cos_sb = pp.tile([128, N], BF16, name="cos_sb")
            sin_sb = pp.tile([128, N], BF16, name="sin_sb")
            mt = pp.tile([128, 128], BF16, name="mt_sb")
            ident = pp.tile([128, 128], BF16, name="id_sb")
            ones1 = pp.tile([1, 128], BF16, name="ones1")
            onesf = pp.tile([1, 128], F32, name="onesf")
            negones = pp.tile([1, 128], F32, name="negones")
            fv = pp.tile([128, QC, H_PER, DIM_HEAD], BF16, name="fv")
            v_aug = pp.tile([128, QC, H_PER, DIM_HEAD + 1], BF16, name="v_aug")
            mix_sb = pp.tile([128, QC, H_PER], BF16, name="mix_sb")
            bmix_sb = pp.tile([H_PER, DEPTH], F32, name="bmix_sb")
            bmix_sb2 = pp.tile([H_PER, DEPTH], F32, name="bmix_sb2")
            binp_sb = pp.tile([128, DEPTH, MI_IN], F32, name="binp_sb")
            bffo_sb = pp.tile([1, DEPTH, DIM], BF16, name="bffo_sb")
            stat = pp.tile([128, 8], F32, name="stat")
            consts = pp.tile([128, 2], F32, name="consts")
            consts05 = pp.tile([H_PER, 1], F32, name="consts05")
            sq_scr = pp.tile([128, DIM], F32, name="sq_scr")
            nc.vector.memset(consts[:, 0:1], 0.0)
            nc.vector.memset(consts[:, 1:2], EPS)
            nc.vector.memset(consts05[:], 0.5)
            nc.const_aps.aps[(F32, 0.0)] = consts[:, 0:1]
            nc.const_aps.aps[(F32, EPS)] = consts[:, 1:2]

            for mi in range(2):
                nc.sync.dma_start(x_sb[:, mi, :], x_in[mi * 128:(mi + 1) * 128, :])
            nc.sync.dma_start(cos_sb[:], ropes[0])
            nc.sync.dma_start(sin_sb[:], ropes[1])
            nc.sync.dma_start(mt[:], mt_in[:])
            nc.sync.dma_start(ident[:], id_in[:])
            nc.sync.dma_start(bmix_sb[:], bmix.rearrange("d h o -> h (d o)"))
            nc.vector.tensor_scalar_mul(bmix_sb2[:], bmix_sb[:], consts05[:, 0:1])
            nc.sync.dma_start(binp_sb[:], b_in.rearrange("d p m -> p d m"))
            nc.sync.dma_start(bffo_sb[:], b_ffo.rearrange("d o f -> o d f"))
            nc.vector.memset(ones1[:], 1.0)
            nc.vector.memset(onesf[:], 1.0)
            nc.vector.memset(negones[:], -1.0)
            nc.vector.memset(v_aug[:, :, :, DIM_HEAD:DIM_HEAD + 1], 1.0)

            def rmsnorm_to_bounce(bounce):
                """x_sb -> rmsnorm -> bf16 rows -> transpose -> DMA bounce."""
                hrow = sp.tile([128, 2, DIM], BF16, tag="hrow")
                sq = sq_scr
                for mi in range(2):
                    s = stat[:, 4 * mi:4 * mi + 4]
                    nc.scalar.activation(sq[:], x_sb[:, mi, :], AF.Square,
                                         accum_out=s[:, 0:1])
                    nc.scalar.activation(s[:, 1:2], s[:, 0:1], AF.Sqrt,
                                         bias=EPS, scale=1.0 / DIM)
                    nc.vector.reciprocal(s[:, 2:3], s[:, 1:2])
                    nc.vector.tensor_scalar_mul(hrow[:, mi, :], x_sb[:, mi, :],
                                                s[:, 2:3])
                hT_st = stp.tile([128, KD, 2 * 128], BF16, tag="hT_st")
                for k in range(KD):
                    for mi in range(2):
                        tp = mmp.tile([128, 128], BF16, tag="mm")
                        nc.tensor.transpose(
                            tp[:], hrow[:, mi, k * 128:(k + 1) * 128], ident[:])
                        nc.vector.tensor_copy(
                            hT_st[:, k, mi * 128:(mi + 1) * 128], tp[:])
                nc.sync.dma_start(
                    bounce.rearrange("(k p) r -> p k r", p=128), hT_st[:])

            def load_hT(bounce_out):
                """Gathered h^T from DRAM -> SBUF [128, k, c, r] in 2 DMAs.

                Layout keeps (c, r) contiguous per k so a global 512-column
                slice (cores 2nj, 2nj+1) is one contiguous run.
                """
                hT = hp.tile([128, KD, N_CORES, ROWS], BF16, tag="hT")
                src4 = bounce_out.rearrange("(c k p) r -> p k c r", c=N_CORES,
                                            k=KD)
                for c in range(N_CORES):
                    nc.sync.dma_start(hT[:, :, c, :], src4[:, :, c, :])
                return hT

            for i in range(DEPTH):
                # ---------------- attention ----------------
                rmsnorm_to_bounce(bh_in[i])
                if local_comm:
                    _local_ag(nc, bh_in[i], bh_out[i], N_CORES)
                else:
                    nc.gpsimd.collective_compute(
                        "AllGather", mybir.AluOpType.bypass, replica_groups=rg,
                        ins=[bh_in[i][:].opt()], outs=[bh_out[i][:].opt()])
                wq_sb = wqp.tile([128, KD, W_QKVM], BF16, tag="wq")
                nc.scalar.dma_start(wq_sb[:], wqkvm[i])
                wo_sb = wop.tile([128, KD, DIM], BF16, tag="wo")
                nc.scalar.dma_start(wo_sb[:], w_o[i])
                hT = load_hT(bh_out[i])

                # qkv + rope + v_aug + attention fused per 512-chunk:
                # attention for q-chunk nj only needs k/v chunks <= nj, so
                # qkv matmuls of later chunks fill the exp-bound bubbles of
                # attention on earlier chunks.
                qT = qkp.tile([128, N], BF16, tag="qT")
                kT = qkp.tile([128, N], BF16, tag="kT")
                vT = qkp.tile([128, N], BF16, tag="vT")
                mixT = qkp.tile([H_PER, N], BF16, tag="mixT")
                oT_loc = qkp.tile([128, N], BF16, tag="oT")
                outs = [qT, kT, vT]
                for nj in range(NJ):
                    nsl = slice(nj * 512, (nj + 1) * 512)
                    # qkv / mix for chunk nj
                    for mi in range(3):
                        ps = mmp.tile([128, 512], F32, tag="mm")
                        for k in range(KD):
                            nc.tensor.matmul(
                                ps[:], wq_sb[:, k, mi * 128:(mi + 1) * 128],
                                hT[:, k, 2 * nj:2 * nj + 2, :],
                                start=(k == 0), stop=(k == KD - 1))
                        if mi == 0:
                            nc.vector.tensor_scalar_mul(qT[:, nsl], ps[:],
                                                        SCALE)
                        else:
                            nc.vector.tensor_copy(outs[mi][:, nsl], ps[:])
                    psm = mmp.tile([H_PER, 512], F32, tag="mm")
                    for k in range(KD):
                        nc.tensor.matmul(psm[:], wq_sb[:, k, 384:384 + H_PER],
                                         hT[:, k, 2 * nj:2 * nj + 2, :],
                                         start=(k == 0), stop=(k == KD - 1))
                    nc.scalar.activation(mixT[:, nsl], psm[:], AF.Tanh,
                                         scale=0.5,
                                         bias=bmix_sb2[:, i:i + 1])
                    # rope for chunk nj (in place)
                    for src_ in (qT, kT):
                        sw = mmp.tile([128, 512], F32, tag="mm")
                        nc.tensor.matmul(sw[:], mt[:], src_[:, nsl],
                                         start=True, stop=True)
                        t1 = sp.tile([128, 512], BF16, tag="ropet1")
                        nc.vector.tensor_mul(t1[:], sw[:], sin_sb[:, nsl])
                        t2 = sp.tile([128, 512], BF16, tag="ropet2")
                        nc.vector.tensor_mul(t2[:], src_[:, nsl],
                                             cos_sb[:, nsl])
                        nc.vector.tensor_add(src_[:, nsl], t1[:], t2[:])
                    # v_aug (+ lerp toward first_v) for t-blocks of chunk nj
                    for t in range(4 * nj, 4 * nj + 4):
                        tp = mmp.tile([128, 128], BF16, tag="mm")
                        nc.tensor.transpose(tp[:],
                                            vT[:, t * 128:(t + 1) * 128],
                                            ident[:])
                        nc.vector.tensor_copy(
                            v_aug[:, t, :, 0:DIM_HEAD],
                            tp[:].rearrange("p (h d) -> p h d", h=H_PER))
                        if i == 0:
                            nc.vector.tensor_copy(fv[:, t],
                                                  v_aug[:, t, :, 0:DIM_HEAD])
                        else:
                            tpm = mmp.tile([128, H_PER], BF16, tag="mm")
                            nc.tensor.transpose(
                                tpm[:], mixT[:, t * 128:(t + 1) * 128],
                                ident[0:H_PER, 0:H_PER])
                            nc.vector.tensor_scalar(
                                mix_sb[:, t, :], tpm[:], 0.5, 0.5,
                                op0=mybir.AluOpType.mult,
                                op1=mybir.AluOpType.add)
                            dl = sp.tile([128, H_PER, DIM_HEAD], BF16,
                                         tag="lerpd")
                            nc.vector.tensor_sub(dl[:], fv[:, t, :, :],
                                                 v_aug[:, t, :, 0:DIM_HEAD])
                            for h in range(H_PER):
                                nc.vector.scalar_tensor_tensor(
                                    v_aug[:, t, h, 0:DIM_HEAD],
                                    dl[:, h, :],
                                    mix_sb[:, t, h:h + 1],
                                    v_aug[:, t, h, 0:DIM_HEAD],
                                    op0=mybir.AluOpType.mult,
                                    op1=mybir.AluOpType.add)
                    # attention for q-chunk qj = nj; K=64 sims of both heads
                    # land on disjoint PE row-groups; AV at N=512 with
                    # above-block-diagonal columns masked to zero; one exp
                    # per t covers both heads.
                    qj = nj
                    qsl = nsl
                    o_ps = psp.tile([DIM_HEAD + 1, 2, 512], F32, tag="ps2",
                                    name=f"ops{i}_{qj}")
                    t_last = 4 * qj + 3
                    for t in range(4 * qj + 4):
                        mcol = max(0, (t - 4 * qj)) * 128
                        sim = psp.tile([128, 2, 512], F32, tag="ps2",
                                       name="sim")
                        for h in range(H_PER):
                            hsl = slice(h * DIM_HEAD, (h + 1) * DIM_HEAD)
                            nc.tensor.matmul(
                                sim[:, h, mcol:512],
                                kT[hsl, t * 128:(t + 1) * 128],
                                qT[hsl, qj * 512 + mcol:(qj + 1) * 512],
                                start=True, stop=True)
                        attn = ap_.tile([128, 2, 512], BF16, tag="attn")
                        if mcol:
                            nc.gpsimd.memset(attn[:, :, 0:mcol], 0.0)
                        nc.scalar.activation(attn[:, :, mcol:512],
                                             sim[:, :, mcol:512], AF.Exp)
                        for h in range(H_PER):
                            nc.tensor.matmul(
                                o_ps[:, h, :], v_aug[:, t, h, :],
                                attn[:, h, :],
                                start=(t == 0), stop=(t == t_last))
                    # normalize columns by 1/den (den = ones-row of v_aug)
                    for h in range(H_PER):
                        hsl = slice(h * DIM_HEAD, (h + 1) * DIM_HEAD)
                        rcp = sp.tile([1, 512], F32, tag="rcp")
                        nc.vector.reciprocal(
                            rcp[:], o_ps[DIM_HEAD:DIM_HEAD + 1, h, :])
                        rcps = sp.tile([DIM_HEAD, 512], F32, tag="rcps")
                        nc.gpsimd.partition_broadcast(rcps[:], rcp[:])
                        nc.vector.tensor_mul(
                            oT_loc[hsl, qsl], o_ps[0:DIM_HEAD, h, :],
                            rcps[:])
                if DEBUG_TAPS and i == 0:
                    nc.sync.dma_start(dbg_qr[:], qrot[:])
                    nc.sync.dma_start(dbg_kr[:], krot[:])
                    nc.sync.dma_start(dbg_oT[:], oT_loc[:])
                # o^T shard-major into the A2A bounce
                nc.sync.dma_start(
                    bo_in[i].rearrange("(j p) r -> p j r", p=128),
                    oT_loc[:].rearrange("p (j r) -> p j r", j=N_CORES))
                if local_comm:
                    nc.sync.dma_start(bo_out[i][:], bo_in[i][:])
                else:
                    nc.gpsimd.collective_compute(
                        "AllToAll", mybir.AluOpType.bypass, replica_groups=rg,
                        ins=[bo_in[i][:].opt()], outs=[bo_out[i][:].opt()])

                # delta_attn = o_full^T[:, rows_c].T @ W_o  (+= into x_sb)
                osl = slp.tile([128, KD, 2, 128], BF16, tag="osl")
                nc.sync.dma_start(
                    osl[:].rearrange("p k m q -> p k (m q)"),
                    bo_out[i].rearrange("(k p) r -> p k r", p=128))
                for mi in range(2):
                    dps = psp.tile([128, 2, 512], F32, tag="ps2",
                                   name=f"dps{mi}")
                    for k in range(KD):
                        for nj2 in range(2):
                            nc.tensor.matmul(
                                dps[:, nj2, :], osl[:, k, mi, :],
                                wo_sb[:, k, nj2 * 512:(nj2 + 1) * 512],
                                start=(k == 0), stop=(k == KD - 1))
                    for nj2 in range(2):
                        nsl = slice(nj2 * 512, (nj2 + 1) * 512)
                        nc.vector.tensor_add(x_sb[:, mi, nsl],
                                             x_sb[:, mi, nsl],
                                             dps[:, nj2, :])

                if DEBUG_TAPS and i == 0:
                    nc.sync.dma_start(
                        dbg_x1[:], x_sb[:].rearrange("p m d -> p (m d)"))
                # ------------- feedforward (sequence-parallel) -------------
                # rmsnorm -> hf rows -> transpose -> hfT_own [128, KD, 256]
                hfrow = sp.tile([128, 2, DIM], BF16, tag="hrow",
                                name=f"hfrow{i}")
                for mi in range(2):
                    s_ = stat[:, 4 * mi:4 * mi + 4]
                    nc.scalar.activation(sq_scr[:], x_sb[:, mi, :], AF.Square,
                                         accum_out=s_[:, 0:1])
                    nc.scalar.activation(s_[:, 1:2], s_[:, 0:1], AF.Sqrt,
                                         bias=EPS, scale=1.0 / DIM)
                    nc.vector.reciprocal(s_[:, 2:3], s_[:, 1:2])
                    nc.vector.tensor_scalar_mul(hfrow[:, mi, :],
                                                x_sb[:, mi, :], s_[:, 2:3])
                hfT = stp.tile([128, KD, 2 * 128], BF16, tag="hT_st",
                               name=f"hfT{i}")
                for k in range(KD):
                    for mi in range(2):
                        tp = mmp.tile([128, 128], BF16, tag="mm",
                                      name=f"hfTt{i}_{k}_{mi}")
                        nc.tensor.transpose(
                            tp[:], hfrow[:, mi, k * 128:(k + 1) * 128],
                            ident[:])
                        nc.vector.tensor_copy(
                            hfT[:, k, mi * 128:(mi + 1) * 128], tp[:])

                # ff_in and ff_out interleaved per hidden chunk: both
                # halves (a|g) of a chunk accumulate into one PSUM bank,
                # and the chunk's ff_out matmuls follow immediately, so the
                # PE stream stays dense through the whole FFN.
                fps = [psp.tile([128, 2, 512], F32, tag="ps2",
                                name=f"fps{_a}") for _a in range(2)]
                for kc in range(KF):
                    rows = FF_ROWS[kc]
                    pps = mmp.tile([128, 2, 256], F32, tag="mm",
                                   name=f"ffps{i}_{kc}")
                    for part in range(2):  # 0 = a, 1 = g
                        ci = part * KF + kc
                        wmi = wip.tile([128, KD, 128], BF16, tag="wi",
                                       name=f"wi{i}_{ci}")
                        nc.scalar.dma_start(wmi[:], w_in[i, ci])
                        for k in range(KD):
                            nc.tensor.matmul(
                                pps[0:rows, part, :], wmi[:, k, 0:rows],
                                hfT[:, k, :], start=(k == 0),
                                stop=(k == KD - 1))
                    g_c = sp.tile([128, 2 * 128], BF16, tag="ffg",
                                  name=f"ffg{i}_{kc}")
                    nc.scalar.activation(
                        g_c[0:rows, :], pps[0:rows, 1, :], AF.Gelu,
                        bias=binp_sb[0:rows, i, KF + kc:KF + kc + 1])
                    act_c = sp.tile([128, 2 * 128], BF16, tag="ffa",
                                    name=f"ffa{i}_{kc}")
                    nc.vector.tensor_scalar_add(
                        act_c[0:rows, :], pps[0:rows, 0, :],
                        binp_sb[0:rows, i, kc:kc + 1])
                    nc.vector.tensor_mul(act_c[0:rows, :], act_c[0:rows, :],
                                         g_c[0:rows, :])
                    wsl = wfp.tile([128, DIM], BF16, tag="wf",
                                   name=f"wf{i}_{kc}")
                    nc.scalar.dma_start(wsl[:], w_ffo[i, :, kc, :])
                    for mi in range(2):
                        for nj2 in range(2):
                            nc.tensor.matmul(
                                fps[mi][:, nj2, :],
                                act_c[0:rows, mi * 128:(mi + 1) * 128],
                                wsl[0:rows, nj2 * 512:(nj2 + 1) * 512],
                                start=(kc == 0), stop=False)
                for mi in range(2):
                    for nj2 in range(2):
                        nsl = slice(nj2 * 512, (nj2 + 1) * 512)
                        nc.tensor.matmul(fps[mi][:, nj2, :], ones1[:, 0:128],
                                         bffo_sb[:, i, nsl], start=False,
                                         stop=True)
                        nc.vector.tensor_add(x_sb[:, mi, nsl],
                                             x_sb[:, mi, nsl],
                                             fps[mi][:, nj2, :])

            # ---------------- final rmsnorm ----------------
            sq = sq_scr
            for mi in range(2):
                s = stat[:, 4 * mi:4 * mi + 4]
                nc.scalar.activation(sq[:], x_sb[:, mi, :], AF.Square,
                                     accum_out=s[:, 0:1])
                nc.scalar.activation(s[:, 1:2], s[:, 0:1], AF.Sqrt,
                                     bias=EPS, scale=1.0 / DIM)
                nc.vector.reciprocal(s[:, 2:3], s[:, 1:2])
                nc.vector.tensor_scalar_mul(sq[:], x_sb[:, mi, :],
                                            s[:, 2:3])
                nc.sync.dma_start(out_ext[mi * 128:(mi + 1) * 128, :],
                                  sq[:])

    nc.compile()
    return nc


_NC_CACHE = {}


def _get_nc():
    if "nc" not in _NC_CACHE:
        _NC_CACHE["nc"] = _build_bass()
    return _NC_CACHE["nc"]


def _rope_tables():
    freqs = 1.0 / (10000.0 ** (np.arange(0, DIM_HEAD, 2, dtype=np.float64)
                               / DIM_HEAD))
    ang = np.arange(N, dtype=np.float64)[:, None] * freqs[None, :]
    cos = np.repeat(np.cos(ang), 2, axis=-1).T  # [64, N]
    sin = np.repeat(np.sin(ang), 2, axis=-1).T
    cos2 = np.concatenate([cos, cos], axis=0)   # [128, N] two heads
    sin2 = np.concatenate([sin, sin], axis=0)
    return np.stack([cos2, sin2]).astype(np.float32)


def _swap_matrix():
    # lhsT for qswap^T = M @ q^T with M[2i,2i+1] = -1, M[2i+1,2i] = +1
    m = np.zeros((DIM_HEAD, DIM_HEAD), np.float32)
    for j in range(DIM_HEAD // 2):
        m[2 * j, 2 * j + 1] = 1.0
        m[2 * j + 1, 2 * j] = -1.0
    mt = np.zeros((128, 128), np.float32)
    mt[0:64, 0:64] = m
    mt[64:128, 64:128] = m
    return mt


def _make_in_maps(tokens, attn_norm_w, w_qkv, w_mix, b_mix, ff_norm_w,
                  w_ff_in, b_ff_in, w_attn_out, w_ff_out, b_ff_out):
    ropes = _bf16(_rope_tables())
    mt = _bf16(_swap_matrix())
    ident = _bf16(np.eye(128, dtype=np.float32))

    # shared (identical on every core) tensors
    wo_l, wf_l, bf_l = [], [], []
    for i in range(DEPTH):
        wo_l.append(w_attn_out[i].reshape(KD, 128, DIM).transpose(1, 0, 2))
        Wf = np.zeros((KF * 128, DIM), np.float32)
        Wf[:FF_HID, :] = w_ff_out[i]
        wf_l.append(Wf.reshape(KF, 128, DIM).transpose(1, 0, 2))
        bf_l.append(b_ff_out[i].reshape(1, DIM))
    w_o_np = _bf16(np.stack(wo_l))
    w_ffo_np = _bf16(np.stack(wf_l))
    b_ffo_np = _bf16(np.stack(bf_l))

    wi_l, bi_l = [], []
    for i in range(DEPTH):
        W = w_ff_in[i] * ff_norm_w[i][:, None]        # [1024, 5460]
        chunks = []
        bp = np.zeros((128, MI_IN), np.float32)
        for part in range(2):
            for kc in range(KF):
                ci = part * KF + kc
                rows = FF_ROWS[kc]
                blk = np.zeros((DIM, 128), np.float32)
                c0 = part * FF_HID + kc * 128
                blk[:, :rows] = W[:, c0:c0 + rows]
                chunks.append(blk.reshape(KD, 128, 128).transpose(1, 0, 2))
                bp[:rows, ci] = b_ff_in[i, c0:c0 + rows]
        wi_l.append(np.stack(chunks))                 # [MI_IN, 128, KD, 128]
        bi_l.append(bp)
    w_in_np = _bf16(np.stack(wi_l))
    b_in_np = np.ascontiguousarray(np.stack(bi_l), dtype=np.float32)

    in_maps = []
    for c in range(N_CORES):
        m = {}
        m["x"] = np.ascontiguousarray(
            tokens[0, c * ROWS:(c + 1) * ROWS, :]).astype(np.float32)
        wq_l, bm_l = [], []
        for i in range(DEPTH):
            W = w_qkv[i] * attn_norm_w[i][:, None]
            Wm = w_mix[i] * attn_norm_w[i][:, None]
            cols = []
            for part in range(3):  # q, k, v
                for h in (2 * c, 2 * c + 1):
                    base = part * HEADS * DIM_HEAD + h * DIM_HEAD
                    cols.append(W[:, base:base + DIM_HEAD])
            cols.append(Wm[:, 2 * c:2 * c + 2])
            Wc = np.concatenate(cols, axis=1)             # [1024, 386]
            wq_l.append(Wc.reshape(KD, 128, W_QKVM).transpose(1, 0, 2))
            bm_l.append(b_mix[i, 2 * c:2 * c + 2].reshape(H_PER, 1))
        m["wqkvm"] = _bf16(np.stack(wq_l))
        m["bmix"] = np.ascontiguousarray(np.stack(bm_l), dtype=np.float32)

        m["w_in"] = w_in_np
        m["b_in"] = b_in_np
        m["w_o"] = w_o_np
        m["w_ffo"] = w_ffo_np
        m["b_ffo"] = b_ffo_np
        m["ropes"] = ropes
        m["mt"] = mt
        m["ident"] = ident
        in_maps.append(m)
    return in_maps


def kernel(tokens, attn_norm_w, w_qkv, w_attn_out, w_mix, b_mix,
           ff_norm_w, w_ff_in, b_ff_in, w_ff_out, b_ff_out, final_norm_w,
           _trace=False):
    tokens = np.asarray(tokens, dtype=np.float32)
    nc = _get_nc()
    in_maps = _make_in_maps(
        tokens,
        np.asarray(attn_norm_w, np.float32), np.asarray(w_qkv, np.float32),
        np.asarray(w_mix, np.float32), np.asarray(b_mix, np.float32),
        np.asarray(ff_norm_w, np.float32), np.asarray(w_ff_in, np.float32),
        np.asarray(b_ff_in, np.float32), np.asarray(w_attn_out, np.float32),
        np.asarray(w_ff_out, np.float32), np.asarray(b_ff_out, np.float32))
    res = run_bass_kernel_spmd(nc, in_maps, core_ids=list(range(N_CORES)),
                               trace=_trace)
    out = np.concatenate([res.results[c]["out"] for c in range(N_CORES)],
                         axis=0)
    out = out * np.asarray(final_norm_w, np.float32)[None, :]
    kernel.last_results = res
    return out.reshape(1, N, DIM).astype(np.float32)



# revision 36
# speedup vs baseline: 1.0161x; 1.0161x over previous
"""Bass/Trainium2 SPMD kernel for nn_BlockCausalTransformer_884763263652.

Sharding over 8 NeuronCores (one chip):
  - Residual x sequence-sharded: core c owns rows [c*256, (c+1)*256).
  - Attention head-sharded: core c owns heads (2c, 2c+1); q/k/v/mix built
    from a column shard of w_qkv (+ w_mix), attention runs fully local.
  - FFN hidden-sharded: hidden padded 2730 -> 2816 = 8*352; core c owns
    slice [c*352,(c+1)*352) of both the a and g halves.
  - Communication per layer: AllGather of h^T (rmsnorm output, bf16) for
    the qkv / ff_in matmuls, and AllToAll of o^T and act^T, which hands
    every core exactly the [contraction, its-rows] slice it needs as the
    stationary operand of the output projections - no reduce collectives
    and no partial sums ever cross cores.
  - Attention is computed in the sim^T = k^T.T @ q^T orientation with an
    extra ones-column appended to v, so softmax needs no transposes and
    the denominator falls out of the same matmul chain.
RMSNorm weights are folded host-side into the following matmul weights;
final_norm_w is applied host-side to the gathered output.
"""

import sys

sys.path.insert(0, "/opt/trn_rl_repo")

import numpy as np
import ml_dtypes

import concourse.bacc as bacc
import concourse.tile as tile
import concourse.mybir as mybir
from concourse.bass_utils import run_bass_kernel_spmd

F32 = mybir.dt.float32
BF16 = mybir.dt.bfloat16
AF = mybir.ActivationFunctionType

DEPTH = 4
DIM = 1024
HEADS = 16
DIM_HEAD = 64
FF_HID = 2730
N = 2048
EPS = 1.1920929e-07
SCALE = DIM_HEAD ** -0.5
N_CORES = 8

ROWS = N // N_CORES            # 256 sequence rows per core
H_PER = HEADS // N_CORES       # 2 heads per core
KD = DIM // 128                # 8 feature chunks
KF = (FF_HID + 127) // 128     # 22 hidden chunks (last ragged: 42 rows)
MI_IN = 2 * KF                 # 44 ff_in M-chunks: a chunks then g chunks
NJ = N // 512                  # 4 sequence 512-chunks
QC = N // 128                  # 16 kpos/qpos 128-chunks
W_QKVM = 3 * H_PER * DIM_HEAD + H_PER  # 386 cols: q|k|v (128 each) + mix (2)
FF_ROWS = [min(128, FF_HID - k * 128) for k in range(KF)]
WI_C = 2                       # ff_in weight stream chunk (column-blocks)
WF_C = 6                       # ff_out weight stream chunk (column-blocks)

_BF = ml_dtypes.bfloat16
DEBUG_TAPS = False


def _bf16(a):
    return np.ascontiguousarray(a.astype(_BF))


def _local_ag(nc, src, dst, n):
    for r in range(n):
        nc.sync.dma_start(dst[r * src.shape[0]:(r + 1) * src.shape[0], :],
                          src[:])


def _build_bass(local_comm=False, n_devices=N_CORES):
    nc = bacc.Bacc("TRN2", target_bir_lowering=False, debug=False,
                   num_devices=n_devices)

    # ---- I/O ----
    x_in = nc.dram_tensor("x", [ROWS, DIM], F32, kind="ExternalInput").ap()
    out_ext = nc.dram_tensor("out", [ROWS, DIM], F32, kind="ExternalOutput").ap()
    wqkvm = nc.dram_tensor("wqkvm", [DEPTH, 128, KD, W_QKVM], BF16,
                           kind="ExternalInput").ap()
    bmix = nc.dram_tensor("bmix", [DEPTH, H_PER, 1], F32,
                          kind="ExternalInput").ap()
    w_in = nc.dram_tensor("w_in", [DEPTH, MI_IN, 128, KD, 128], BF16,
                          kind="ExternalInput").ap()
    b_in = nc.dram_tensor("b_in", [DEPTH, 128, MI_IN], F32,
                          kind="ExternalInput").ap()
    w_o = nc.dram_tensor("w_o", [DEPTH, 128, KD, DIM], BF16,
                         kind="ExternalInput").ap()
    w_ffo = nc.dram_tensor("w_ffo", [DEPTH, 128, KF, DIM], BF16,
                           kind="ExternalInput").ap()
    b_ffo = nc.dram_tensor("b_ffo", [DEPTH, 1, DIM], BF16,
                           kind="ExternalInput").ap()
    ropes = nc.dram_tensor("ropes", [2, 128, N], BF16,
                           kind="ExternalInput").ap()
    mt_in = nc.dram_tensor("mt", [128, 128], BF16, kind="ExternalInput").ap()
    id_in = nc.dram_tensor("ident", [128, 128], BF16, kind="ExternalInput").ap()
    if DEBUG_TAPS:
        dbg_qr = nc.dram_tensor("dbg_qr", [128, N], BF16,
                                kind="ExternalOutput").ap()
        dbg_kr = nc.dram_tensor("dbg_kr", [128, N], BF16,
                                kind="ExternalOutput").ap()
        dbg_oT = nc.dram_tensor("dbg_oT", [128, N], BF16,
                                kind="ExternalOutput").ap()
        dbg_x1 = nc.dram_tensor("dbg_x1", [128, 2 * DIM], F32,
                                kind="ExternalOutput").ap()

    # ---- per-layer DRAM bounce buffers for collectives ----
    rg = [list(range(N_CORES))]
    bh_in = [nc.dram_tensor(f"hin{j}", [DIM, ROWS], BF16).ap()
             for j in range(DEPTH)]
    bh_out = [nc.dram_tensor(f"hout{j}", [N_CORES * DIM, ROWS], BF16,
                             addr_space="Shared").ap()
              for j in range(DEPTH)]
    bo_in = [nc.dram_tensor(f"oin{i}", [N_CORES * 128, ROWS], BF16).ap()
             for i in range(DEPTH)]
    bo_out = [nc.dram_tensor(f"oout{i}", [N_CORES * 128, ROWS], BF16).ap()
              for i in range(DEPTH)]

    with tile.TileContext(nc) as tc:
        with (
            tc.tile_pool(name="persist", bufs=1) as pp,
            tc.tile_pool(name="wq", bufs=2) as wqp,
            tc.tile_pool(name="wi", bufs=6) as wip,
            tc.tile_pool(name="wo", bufs=1) as wop,
            tc.tile_pool(name="wf", bufs=6) as wfp,
            tc.tile_pool(name="hT", bufs=1) as hp,
            tc.tile_pool(name="qk", bufs=1) as qkp,
            tc.tile_pool(name="attn", bufs=4) as ap_,
            tc.tile_pool(name="scratch", bufs=2) as sp,
            tc.tile_pool(name="stage", bufs=1) as stp,
            tc.tile_pool(name="slice", bufs=1) as slp,
            tc.tile_pool(name="mm", bufs=2, space="PSUM") as mmp,
            tc.tile_pool(name="ps2", bufs=3, space="PSUM") as psp,
        ):
            # persistent tiles
            x_sb = pp.tile([128, 2, DIM], F32, name="x_sb")
            cos_sb = pp.tile([128, N], BF16, name="cos_sb")
            sin_sb = pp.tile([128, N], BF16, name="sin_sb")
            mt = pp.tile([128, 128], BF16, name="mt_sb")
            ident = pp.tile([128, 128], BF16, name="id_sb")
            ones1 = pp.tile([1, 128], BF16, name="ones1")
            onesf = pp.tile([1, 128], F32, name="onesf")
            negones = pp.tile([1, 128], F32, name="negones")
            fv = pp.tile([128, QC, H_PER, DIM_HEAD], BF16, name="fv")
            v_aug = pp.tile([128, QC, H_PER, DIM_HEAD + 1], BF16, name="v_aug")
            mix_sb = pp.tile([128, QC, H_PER], BF16, name="mix_sb")
            bmix_sb = pp.tile([H_PER, DEPTH], F32, name="bmix_sb")
            bmix_sb2 = pp.tile([H_PER, DEPTH], F32, name="bmix_sb2")
            binp_sb = pp.tile([128, DEPTH, MI_IN], F32, name="binp_sb")
            bffo_sb = pp.tile([1, DEPTH, DIM], BF16, name="bffo_sb")
            stat = pp.tile([128, 8], F32, name="stat")
            consts = pp.tile([128, 2], F32, name="consts")
            consts05 = pp.tile([H_PER, 1], F32, name="consts05")
            sq_scr = pp.tile([128, DIM], F32, name="sq_scr")
            nc.vector.memset(consts[:, 0:1], 0.0)
            nc.vector.memset(consts[:, 1:2], EPS)
            nc.vector.memset(consts05[:], 0.5)
            nc.const_aps.aps[(F32, 0.0)] = consts[:, 0:1]
            nc.const_aps.aps[(F32, EPS)] = consts[:, 1:2]

            for mi in range(2):
                nc.sync.dma_start(x_sb[:, mi, :], x_in[mi * 128:(mi + 1) * 128, :])
            nc.sync.dma_start(cos_sb[:], ropes[0])
            nc.sync.dma_start(sin_sb[:], ropes[1])
            nc.sync.dma_start(mt[:], mt_in[:])
            nc.sync.dma_start(ident[:], id_in[:])
            nc.sync.dma_start(bmix_sb[:], bmix.rearrange("d h o -> h (d o)"))
            nc.vector.tensor_scalar_mul(bmix_sb2[:], bmix_sb[:], consts05[:, 0:1])
            nc.sync.dma_start(binp_sb[:], b_in.rearrange("d p m -> p d m"))
            nc.sync.dma_start(bffo_sb[:], b_ffo.rearrange("d o f -> o d f"))
            nc.vector.memset(ones1[:], 1.0)
            nc.vector.memset(onesf[:], 1.0)
            nc.vector.memset(negones[:], -1.0)
            nc.vector.memset(v_aug[:, :, :, DIM_HEAD:DIM_HEAD + 1], 1.0)

            def rmsnorm_to_bounce(bounce):
                """x_sb -> rmsnorm -> bf16 rows -> transpose -> DMA bounce."""
                hrow = sp.tile([128, 2, DIM], BF16, tag="hrow")
                sq = sq_scr
                for mi in range(2):
                    s = stat[:, 4 * mi:4 * mi + 4]
                    nc.scalar.activation(sq[:], x_sb[:, mi, :], AF.Square,
                                         accum_out=s[:, 0:1])
                    nc.scalar.activation(s[:, 1:2], s[:, 0:1], AF.Sqrt,
                                         bias=EPS, scale=1.0 / DIM)
                    nc.vector.reciprocal(s[:, 2:3], s[:, 1:2])
                    nc.vector.tensor_scalar_mul(hrow[:, mi, :], x_sb[:, mi, :],
                                                s[:, 2:3])
                hT_st = stp.tile([128, KD, 2 * 128], BF16, tag="hT_st")
                for k in range(KD):
                    for mi in range(2):
                        tp = mmp.tile([128, 128], BF16, tag="mm")
                        nc.tensor.transpose(
                            tp[:], hrow[:, mi, k * 128:(k + 1) * 128], ident[:])
                        nc.vector.tensor_copy(
                            hT_st[:, k, mi * 128:(mi + 1) * 128], tp[:])
                nc.sync.dma_start(
                    bounce.rearrange("(k p) r -> p k r", p=128), hT_st[:])

            def load_hT(bounce_out):
                """Gathered h^T from DRAM -> SBUF [128, k, c, r] in 2 DMAs.

                Layout keeps (c, r) contiguous per k so a global 512-column
                slice (cores 2nj, 2nj+1) is one contiguous run.
                """
                hT = hp.tile([128, KD, N_CORES, ROWS], BF16, tag="hT")
                src4 = bounce_out.rearrange("(c k p) r -> p k c r", c=N_CORES,
                                            k=KD)
                for c in range(N_CORES):
                    nc.sync.dma_start(hT[:, :, c, :], src4[:, :, c, :])
                return hT

            for i in range(DEPTH):
                # ---------------- attention ----------------
                rmsnorm_to_bounce(bh_in[i])
                if local_comm:
                    _local_ag(nc, bh_in[i], bh_out[i], N_CORES)
                else:
                    nc.gpsimd.collective_compute(
                        "AllGather", mybir.AluOpType.bypass, replica_groups=rg,
                        ins=[bh_in[i][:].opt()], outs=[bh_out[i][:].opt()])
                wq_sb = wqp.tile([128, KD, W_QKVM], BF16, tag="wq")
                nc.scalar.dma_start(wq_sb[:], wqkvm[i])
                wo_sb = wop.tile([128, KD, DIM], BF16, tag="wo")
                nc.scalar.dma_start(wo_sb[:], w_o[i])
                hT = load_hT(bh_out[i])

                # qkv + rope + v_aug + attention fused per 512-chunk:
                # attention for q-chunk nj only needs k/v chunks <= nj, so
                # qkv matmuls of later chunks fill the exp-bound bubbles of
                # attention on earlier chunks.
                qT = qkp.tile([128, N], BF16, tag="qT")
                kT = qkp.tile([128, N], BF16, tag="kT")
                vT = qkp.tile([128, N], BF16, tag="vT")
                mixT = qkp.tile([H_PER, N], BF16, tag="mixT")
                oT_loc = qkp.tile([128, N], BF16, tag="oT")
                outs = [qT, kT, vT]
                for nj in range(NJ):
                    nsl = slice(nj * 512, (nj + 1) * 512)
                    # qkv / mix for chunk nj
                    for mi in range(3):
                        ps = mmp.tile([128, 512], F32, tag="mm")
                        for k in range(KD):
                            nc.tensor.matmul(
                                ps[:], wq_sb[:, k, mi * 128:(mi + 1) * 128],
                                hT[:, k, 2 * nj:2 * nj + 2, :],
                                start=(k == 0), stop=(k == KD - 1))
                        if mi == 0:
                            nc.vector.tensor_scalar_mul(qT[:, nsl], ps[:],
                                                        SCALE)
                        else:
                            nc.vector.tensor_copy(outs[mi][:, nsl], ps[:])
                    psm = mmp.tile([H_PER, 512], F32, tag="mm")
                    for k in range(KD):
                        nc.tensor.matmul(psm[:], wq_sb[:, k, 384:384 + H_PER],
                                         hT[:, k, 2 * nj:2 * nj + 2, :],
                                         start=(k == 0), stop=(k == KD - 1))
                    nc.scalar.activation(mixT[:, nsl], psm[:], AF.Tanh,
                                         scale=0.5,
                                         bias=bmix_sb2[:, i:i + 1])
                    # rope for chunk nj (in place)
                    for src_ in (qT, kT):
                        sw = mmp.tile([128, 512], F32, tag="mm")
                        nc.tensor.matmul(sw[:], mt[:], src_[:, nsl],
                                         start=True, stop=True)
                        t1 = sp.tile([128, 512], BF16, tag="ropet1")
                        nc.vector.tensor_mul(t1[:], sw[:], sin_sb[:, nsl])
                        t2 = sp.tile([128, 512], BF16, tag="ropet2")
                        nc.vector.tensor_mul(t2[:], src_[:, nsl],
                                             cos_sb[:, nsl])
                        nc.vector.tensor_add(src_[:, nsl], t1[:], t2[:])
                    # v_aug (+ lerp toward first_v) for t-blocks of chunk nj
                    for t in range(4 * nj, 4 * nj + 4):
                        tp = mmp.tile([128, 128], BF16, tag="mm")
                        nc.tensor.transpose(tp[:],
                                            vT[:, t * 128:(t + 1) * 128],
                                            ident[:])
                        nc.vector.tensor_copy(
                            v_aug[:, t, :, 0:DIM_HEAD],
                            tp[:].rearrange("p (h d) -> p h d", h=H_PER))
                        if i == 0:
                            nc.vector.tensor_copy(fv[:, t],
                                                  v_aug[:, t, :, 0:DIM_HEAD])
                        else:
                            tpm = mmp.tile([128, H_PER], BF16, tag="mm")
                            nc.tensor.transpose(
                                tpm[:], mixT[:, t * 128:(t + 1) * 128],
                                ident[0:H_PER, 0:H_PER])
                            nc.vector.tensor_scalar(
                                mix_sb[:, t, :], tpm[:], 0.5, 0.5,
                                op0=mybir.AluOpType.mult,
                                op1=mybir.AluOpType.add)
                            dl = sp.tile([128, H_PER, DIM_HEAD], BF16,
                                         tag="lerpd")
                            nc.vector.tensor_sub(dl[:], fv[:, t, :, :],
                                                 v_aug[:, t, :, 0:DIM_HEAD])
                            for h in range(H_PER):
                                nc.vector.scalar_tensor_tensor(
                                    v_aug[:, t, h, 0:DIM_HEAD],
                                    dl[:, h, :],
                                    mix_sb[:, t, h:h + 1],
                                    v_aug[:, t, h, 0:DIM_HEAD],
                                    op0=mybir.AluOpType.mult,
                                    op1=mybir.AluOpType.add)
                    # attention for q-chunk qj = nj; K=64 sims of both heads
                    # land on disjoint PE row-groups; AV at N=512 with
                    # above-block-diagonal columns masked to zero; one exp
                    # per t covers both heads.
                    qj = nj
                    qsl = nsl
                    o_ps = psp.tile([DIM_HEAD + 1, 2, 512], F32, tag="ps2",
                                    name=f"ops{i}_{qj}")
                    t_last = 4 * qj + 3
                    for t in range(4 * qj + 4):
                        mcol = max(0, (t - 4 * qj)) * 128
                        sim = psp.tile([128, 2, 512], F32, tag="ps2",
                                       name="sim")
                        for h in range(H_PER):
                            hsl = slice(h * DIM_HEAD, (h + 1) * DIM_HEAD)
                            nc.tensor.matmul(
                                sim[:, h, mcol:512],
                                kT[hsl, t * 128:(t + 1) * 128],
                                qT[hsl, qj * 512 + mcol:(qj + 1) * 512],
                                start=True, stop=True)
                        attn = ap_.tile([128, 2, 512], BF16, tag="attn")
                        if mcol:
                            nc.vector.memset(attn[:, :, 0:mcol], 0.0)
                        nc.scalar.activation(attn[:, :, mcol:512],
                                             sim[:, :, mcol:512], AF.Exp)
                        for h in range(H_PER):
                            nc.tensor.matmul(
                                o_ps[:, h, :], v_aug[:, t, h, :],
                                attn[:, h, :],
                                start=(t == 0), stop=(t == t_last))
                    # normalize columns by 1/den (den = ones-row of v_aug)
                    for h in range(H_PER):
                        hsl = slice(h * DIM_HEAD, (h + 1) * DIM_HEAD)
                        rcp = sp.tile([1, 512], F32, tag="rcp")
                        nc.vector.reciprocal(
                            rcp[:], o_ps[DIM_HEAD:DIM_HEAD + 1, h, :])
                        rcps = sp.tile([DIM_HEAD, 512], F32, tag="rcps")
                        nc.gpsimd.partition_broadcast(rcps[:], rcp[:])
                        nc.vector.tensor_mul(
                            oT_loc[hsl, qsl], o_ps[0:DIM_HEAD, h, :],
                            rcps[:])
                if DEBUG_TAPS and i == 0:
                    nc.sync.dma_start(dbg_qr[:], qrot[:])
                    nc.sync.dma_start(dbg_kr[:], krot[:])
                    nc.sync.dma_start(dbg_oT[:], oT_loc[:])
                # o^T shard-major into the A2A bounce
                nc.sync.dma_start(
                    bo_in[i].rearrange("(j p) r -> p j r", p=128),
                    oT_loc[:].rearrange("p (j r) -> p j r", j=N_CORES))
                if local_comm:
                    nc.sync.dma_start(bo_out[i][:], bo_in[i][:])
                else:
                    nc.gpsimd.collective_compute(
                        "AllToAll", mybir.AluOpType.bypass, replica_groups=rg,
                        ins=[bo_in[i][:].opt()], outs=[bo_out[i][:].opt()])

                # delta_attn = o_full^T[:, rows_c].T @ W_o  (+= into x_sb)
                osl = slp.tile([128, KD, 2, 128], BF16, tag="osl")
                nc.sync.dma_start(
                    osl[:].rearrange("p k m q -> p k (m q)"),
                    bo_out[i].rearrange("(k p) r -> p k r", p=128))
                for mi in range(2):
                    dps = psp.tile([128, 2, 512], F32, tag="ps2",
                                   name=f"dps{mi}")
                    for k in range(KD):
                        for nj2 in range(2):
                            nc.tensor.matmul(
                                dps[:, nj2, :], osl[:, k, mi, :],
                                wo_sb[:, k, nj2 * 512:(nj2 + 1) * 512],
                                start=(k == 0), stop=(k == KD - 1))
                    for nj2 in range(2):
                        nsl = slice(nj2 * 512, (nj2 + 1) * 512)
                        nc.vector.tensor_add(x_sb[:, mi, nsl],
                                             x_sb[:, mi, nsl],
                                             dps[:, nj2, :])

                if DEBUG_TAPS and i == 0:
                    nc.sync.dma_start(
                        dbg_x1[:], x_sb[:].rearrange("p m d -> p (m d)"))
                # ------------- feedforward (sequence-parallel) -------------
                # rmsnorm -> hf rows -> transpose -> hfT_own [128, KD, 256]
                hfrow = sp.tile([128, 2, DIM], BF16, tag="hrow",
                                name=f"hfrow{i}")
                for mi in range(2):
                    s_ = stat[:, 4 * mi:4 * mi + 4]
                    nc.scalar.activation(sq_scr[:], x_sb[:, mi, :], AF.Square,
                                         accum_out=s_[:, 0:1])
                    nc.scalar.activation(s_[:, 1:2], s_[:, 0:1], AF.Sqrt,
                                         bias=EPS, scale=1.0 / DIM)
                    nc.vector.reciprocal(s_[:, 2:3], s_[:, 1:2])
                    nc.vector.tensor_scalar_mul(hfrow[:, mi, :],
                                                x_sb[:, mi, :], s_[:, 2:3])
                hfT = stp.tile([128, KD, 2 * 128], BF16, tag="hT_st",
                               name=f"hfT{i}")
                for k in range(KD):
                    for mi in range(2):
                        tp = mmp.tile([128, 128], BF16, tag="mm",
                                      name=f"hfTt{i}_{k}_{mi}")
                        nc.tensor.transpose(
                            tp[:], hfrow[:, mi, k * 128:(k + 1) * 128],
                            ident[:])
                        nc.vector.tensor_copy(
                            hfT[:, k, mi * 128:(mi + 1) * 128], tp[:])

                # ff_in and ff_out interleaved per hidden chunk: both
                # halves (a|g) of a chunk accumulate into one PSUM bank,
                # and the chunk's ff_out matmuls follow immediately, so the
                # PE stream stays dense through the whole FFN.
                fps = [psp.tile([128, 2, 512], F32, tag="ps2",
                                name=f"fps{_a}") for _a in range(2)]
                for kc in range(KF):
                    rows = FF_ROWS[kc]
                    pps = mmp.tile([128, 2, 256], F32, tag="mm",
                                   name=f"ffps{i}_{kc}")
                    for part in range(2):  # 0 = a, 1 = g
                        ci = part * KF + kc
                        wmi = wip.tile([128, KD, 128], BF16, tag="wi",
                                       name=f"wi{i}_{ci}")
                        nc.scalar.dma_start(wmi[:], w_in[i, ci])
                        for k in range(KD):
                            nc.tensor.matmul(
                                pps[0:rows, part, :], wmi[:, k, 0:rows],
                                hfT[:, k, :], start=(k == 0),
                                stop=(k == KD - 1))
                    g_c = sp.tile([128, 2 * 128], BF16, tag="ffg",
                                  name=f"ffg{i}_{kc}")
                    nc.scalar.activation(
                        g_c[0:rows, :], pps[0:rows, 1, :], AF.Gelu,
                        bias=binp_sb[0:rows, i, KF + kc:KF + kc + 1])
                    act_c = sp.tile([128, 2 * 128], BF16, tag="ffa",
                                    name=f"ffa{i}_{kc}")
                    nc.vector.tensor_scalar_add(
                        act_c[0:rows, :], pps[0:rows, 0, :],
                        binp_sb[0:rows, i, kc:kc + 1])
                    nc.vector.tensor_mul(act_c[0:rows, :], act_c[0:rows, :],
                                         g_c[0:rows, :])
                    wsl = wfp.tile([128, DIM], BF16, tag="wf",
                                   name=f"wf{i}_{kc}")
                    nc.scalar.dma_start(wsl[:], w_ffo[i, :, kc, :])
                    for mi in range(2):
                        for nj2 in range(2):
                            nc.tensor.matmul(
                                fps[mi][:, nj2, :],
                                act_c[0:rows, mi * 128:(mi + 1) * 128],
                                wsl[0:rows, nj2 * 512:(nj2 + 1) * 512],
                                start=(kc == 0), stop=False)
                for mi in range(2):
                    for nj2 in range(2):
                        nsl = slice(nj2 * 512, (nj2 + 1) * 512)
                        nc.tensor.matmul(fps[mi][:, nj2, :], ones1[:, 0:128],
                                         bffo_sb[:, i, nsl], start=False,
                                         stop=True)
                        nc.vector.tensor_add(x_sb[:, mi, nsl],
                                             x_sb[:, mi, nsl],
                                             fps[mi][:, nj2, :])

            # ---------------- final rmsnorm ----------------
            sq = sq_scr
            for mi in range(2):
                s = stat[:, 4 * mi:4 * mi + 4]
                nc.scalar.activation(sq[:], x_sb[:, mi, :], AF.Square,
                                     accum_out=s[:, 0:1])
                nc.scalar.activation(s[:, 1:2], s[:, 0:1], AF.Sqrt,
                                     bias=EPS, scale=1.0 / DIM)
                nc.vector.reciprocal(s[:, 2:3], s[:, 1:2])
                nc.vector.tensor_scalar_mul(sq[:], x_sb[:, mi, :],
                                            s[:, 2:3])
                nc.sync.dma_start(out_ext[mi * 128:(mi + 1) * 128, :],
                                  sq[:])

    nc.compile()
    return nc


_NC_CACHE = {}


def _get_nc():
    if "nc" not in _NC_CACHE:
        _NC_CACHE["nc"] = _build_bass()
    return _NC_CACHE["nc"]


def _rope_tables():
    freqs = 1.0 / (10000.0 ** (np.arange(0, DIM_HEAD, 2, dtype=np.float64)
                               / DIM_HEAD))
    ang = np.arange(N, dtype=np.float64)[:, None] * freqs[None, :]
    cos = np.repeat(np.cos(ang), 2, axis=-1).T  # [64, N]
    sin = np.repeat(np.sin(ang), 2, axis=-1).T
    cos2 = np.concatenate([cos, cos], axis=0)   # [128, N] two heads
    sin2 = np.concatenate([sin, sin], axis=0)
    return np.stack([cos2, sin2]).astype(np.float32)


def _swap_matrix():
    # lhsT for qswap^T = M @ q^T with M[2i,2i+1] = -1, M[2i+1,2i] = +1
    m = np.zeros((DIM_HEAD, DIM_HEAD), np.float32)
    for j in range(DIM_HEAD // 2):
        m[2 * j, 2 * j + 1] = 1.0
        m[2 * j + 1, 2 * j] = -1.0
    mt = np.zeros((128, 128), np.float32)
    mt[0:64, 0:64] = m
    mt[64:128, 64:128] = m
    return mt


def _make_in_maps(tokens, attn_norm_w, w_qkv, w_mix, b_mix, ff_norm_w,
                  w_ff_in, b_ff_in, w_attn_out, w_ff_out, b_ff_out):
    ropes = _bf16(_rope_tables())
    mt = _bf16(_swap_matrix())
    ident = _bf16(np.eye(128, dtype=np.float32))

    # shared (identical on every core) tensors
    wo_l, wf_l, bf_l = [], [], []
    for i in range(DEPTH):
        wo_l.append(w_attn_out[i].reshape(KD, 128, DIM).transpose(1, 0, 2))
        Wf = np.zeros((KF * 128, DIM), np.float32)
        Wf[:FF_HID, :] = w_ff_out[i]
        wf_l.append(Wf.reshape(KF, 128, DIM).transpose(1, 0, 2))
        bf_l.append(b_ff_out[i].reshape(1, DIM))
    w_o_np = _bf16(np.stack(wo_l))
    w_ffo_np = _bf16(np.stack(wf_l))
    b_ffo_np = _bf16(np.stack(bf_l))

    wi_l, bi_l = [], []
    for i in range(DEPTH):
        W = w_ff_in[i] * ff_norm_w[i][:, None]        # [1024, 5460]
        chunks = []
        bp = np.zeros((128, MI_IN), np.float32)
        for part in range(2):
            for kc in range(KF):
                ci = part * KF + kc
                rows = FF_ROWS[kc]
                blk = np.zeros((DIM, 128), np.float32)
                c0 = part * FF_HID + kc * 128
                blk[:, :rows] = W[:, c0:c0 + rows]
                chunks.append(blk.reshape(KD, 128, 128).transpose(1, 0, 2))
                bp[:rows, ci] = b_ff_in[i, c0:c0 + rows]
        wi_l.append(np.stack(chunks))                 # [MI_IN, 128, KD, 128]
        bi_l.append(bp)
    w_in_np = _bf16(np.stack(wi_l))
    b_in_np = np.ascontiguousarray(np.stack(bi_l), dtype=np.float32)

    in_maps = []
    for c in range(N_CORES):
        m = {}
        m["x"] = np.ascontiguousarray(
            tokens[0, c * ROWS:(c + 1) * ROWS, :]).astype(np.float32)
        wq_l, bm_l = [], []
        for i in range(DEPTH):
            W = w_qkv[i] * attn_norm_w[i][:, None]
            Wm = w_mix[i] * attn_norm_w[i][:, None]
            cols = []
            for part in range(3):  # q, k, v
                for h in (2 * c, 2 * c + 1):
                    base = part * HEADS * DIM_HEAD + h * DIM_HEAD
                    cols.append(W[:, base:base + DIM_HEAD])
            cols.append(Wm[:, 2 * c:2 * c + 2])
            Wc = np.concatenate(cols, axis=1)             # [1024, 386]
            wq_l.append(Wc.reshape(KD, 128, W_QKVM).transpose(1, 0, 2))
            bm_l.append(b_mix[i, 2 * c:2 * c + 2].reshape(H_PER, 1))
        m["wqkvm"] = _bf16(np.stack(wq_l))
        m["bmix"] = np.ascontiguousarray(np.stack(bm_l), dtype=np.float32)

        m["w_in"] = w_in_np
        m["b_in"] = b_in_np
        m["w_o"] = w_o_np
        m["w_ffo"] = w_ffo_np
        m["b_ffo"] = b_ffo_np
        m["ropes"] = ropes
        m["mt"] = mt
        m["ident"] = ident
        in_maps.append(m)
    return in_maps


def kernel(tokens, attn_norm_w, w_qkv, w_attn_out, w_mix, b_mix,
           ff_norm_w, w_ff_in, b_ff_in, w_ff_out, b_ff_out, final_norm_w,
           _trace=False):
    tokens = np.asarray(tokens, dtype=np.float32)
    nc = _get_nc()
    in_maps = _make_in_maps(
        tokens,
        np.asarray(attn_norm_w, np.float32), np.asarray(w_qkv, np.float32),
        np.asarray(w_mix, np.float32), np.asarray(b_mix, np.float32),
        np.asarray(ff_norm_w, np.float32), np.asarray(w_ff_in, np.float32),
        np.asarray(b_ff_in, np.float32), np.asarray(w_attn_out, np.float32),
        np.asarray(w_ff_out, np.float32), np.asarray(b_ff_out, np.float32))
    res = run_bass_kernel_spmd(nc, in_maps, core_ids=list(range(N_CORES)),
                               trace=_trace)
    out = np.concatenate([res.results[c]["out"] for c in range(N_CORES)],
                         axis=0)
    out = out * np.asarray(final_norm_w, np.float32)[None, :]
    kernel.last_results = res
    return out.reshape(1, N, DIM).astype(np.float32)

